# revision 1
# baseline (speedup 1.0000x reference)
"""BaseAttentionPooling Trainium2 kernel.

reference:
    h = tanh(x @ W1 + b1)            # [N, H]
    logits = (h @ W2 + b2)[:, 0]     # [N]
    per-graph softmax over sorted `batch`, pooled = seg_sum(x * w)  # [G, D]

Strategy (data-parallel over graphs, 8 cores, SPMD-identical program):
  - 512 graphs/core, split into 4 blocks of 128 graphs.
  - Host pads each (core, block)'s nodes to `cpb` chunks of 128 nodes
    (cpb = max over all core/blocks, so the program is core-uniform).
  - b2 is dropped: it cancels in the softmax.
  - exp() is computed without max-subtraction: |logits| <= ||W2||_1 + |b2|
    is small (~6), so exp() is safe in fp32.
  - Per chunk: PE transpose x -> hT = W1.T @ xT (bf16, f32 accum) ->
    tanh(+b1) on ACT -> logits = hT.T @ W2 (PE, N=1) -> batched exp ->
    DVE builds scaled_onehot[i, g] = (iota == rel_gid[i]) * e[i] ->
    PE accumulates pooled[g, 0:256] += onehot.T @ x and denom[g] += onehot.T @ 1
    in PSUM across all chunks of the block.
  - Epilogue: pooled / max(denom, tiny), DMA out.
"""

import os
import sys

import numpy as np

for _p in ("/opt/trn_rl_repo",):
    if _p not in sys.path and os.path.isdir(_p):
        sys.path.insert(0, _p)

import ml_dtypes

import concourse.bass as bass
import concourse.tile as tile
from concourse import bacc, mybir
from concourse import bass_utils

N, D, H, G = 500000, 256, 128, 4096
NCORES = 8
GPC = G // NCORES          # graphs per core = 512
NBLK = 4                   # graph-blocks per core
BLKG = GPC // NBLK         # graphs per block = 128
P = 128                    # partition / chunk size

BF16 = mybir.dt.bfloat16
F32 = mybir.dt.float32
NP_BF16 = ml_dtypes.bfloat16

LAST_RESULT = None  # test.py reads exec_time_ns / profile from here
DEBUG_TAPS = False  # dump first-group intermediates as extra outputs


# ---------------------------------------------------------------- host plan

def make_plan(batch):
    """Compute the uniform chunk layout from the sorted graph ids."""
    batch = np.asarray(batch)
    seg = np.searchsorted(batch, np.arange(G + 1), side="left")  # [G+1]
    counts = np.zeros((NCORES, NBLK), dtype=np.int64)
    for c in range(NCORES):
        for b in range(NBLK):
            g0 = c * GPC + b * BLKG
            counts[c, b] = seg[g0 + BLKG] - seg[g0]
    cpb = int(np.ceil(counts.max() / P))
    if cpb % 2:
        cpb += 1               # CH = 4*cpb must be divisible by 8
    ch = NBLK * cpb            # chunks per core
    return seg, counts, cpb, ch


def build_inputs(x, batch, W1, b1, W2, seg, cpb, ch):
    """Build the 8 per-core input maps (all shipped data)."""
    x = np.asarray(x)
    batch = np.asarray(batch)
    x_bf = x.astype(NP_BF16)
    w1_bf = np.asarray(W1).astype(NP_BF16)
    b1_f = np.asarray(b1).astype(np.float32).reshape(H, 1)
    w2_f = np.asarray(W2).astype(NP_BF16).reshape(H, 1)
    ident = np.eye(P, dtype=NP_BF16)
    iota = np.broadcast_to(
        np.arange(P, dtype=np.float32), (P, P)
    ).astype(NP_BF16)  # iota[p, f] = f (0..127 exact in bf16)

    in_maps = []
    for c in range(NCORES):
        xs = np.zeros((ch * P, D), dtype=NP_BF16)
        rel = np.full(ch * P, -1.0, dtype=np.float32)
        for b in range(NBLK):
            g0 = c * GPC + b * BLKG
            s0, s1 = int(seg[g0]), int(seg[g0 + BLKG])
            n = s1 - s0
            r0 = b * cpb * P
            xs[r0 : r0 + n] = x_bf[s0:s1]
            rel[r0 : r0 + n] = (batch[s0:s1] - g0).astype(np.float32)
        blr = np.ascontiguousarray(rel.reshape(ch, P).T)  # [128, CH] f32
        in_maps.append(
            {
                "xs": xs,
                "blr": blr,
                "w1": w1_bf,
                "b1": b1_f,
                "w2": w2_f,
                "ident": ident,
                "iota": iota,
            }
        )
    return in_maps


# ------------------------------------------------------------- bass program

def build_bass(ch, cpb):
    """Build the SPMD-uniform per-core program."""
    nc = bacc.Bacc(
        "TRN2",
        target_bir_lowering=False,
        debug=False,
        num_devices=NCORES,
    )
    xs = nc.dram_tensor("xs", [ch * P, D], BF16, kind="ExternalInput").ap()
    blr = nc.dram_tensor("blr", [P, ch], F32, kind="ExternalInput").ap()
    w1 = nc.dram_tensor("w1", [D, H], BF16, kind="ExternalInput").ap()
    b1 = nc.dram_tensor("b1", [H, 1], F32, kind="ExternalInput").ap()
    w2 = nc.dram_tensor("w2", [H, 1], BF16, kind="ExternalInput").ap()
    ident = nc.dram_tensor("ident", [P, P], BF16, kind="ExternalInput").ap()
    iota = nc.dram_tensor("iota", [P, P], BF16, kind="ExternalInput").ap()
    out = nc.dram_tensor("out", [GPC, D], F32, kind="ExternalOutput").ap()
    dbg = {}
    if DEBUG_TAPS:
        dbg["xb"] = nc.dram_tensor("dbg_xb", [P, 8 * D], BF16, kind="ExternalOutput").ap()
        dbg["xt"] = nc.dram_tensor("dbg_xt", [P, 4 * D], BF16, kind="ExternalOutput").ap()
        dbg["e"] = nc.dram_tensor("dbg_e", [P, 8], F32, kind="ExternalOutput").ap()
        dbg["lg"] = nc.dram_tensor("dbg_lg", [P, 8], F32, kind="ExternalOutput").ap()
        dbg["oh"] = nc.dram_tensor("dbg_oh", [P, P], BF16, kind="ExternalOutput").ap()

    n_g8 = ch // 8  # 8-chunk groups

    with tile.TileContext(nc) as tc:
        with (
            tc.tile_pool(name="consts", bufs=1) as cpool,
            tc.tile_pool(name="xb", bufs=4) as xbpool,
            tc.tile_pool(name="xtsb", bufs=4) as xtsbpool,
            tc.tile_pool(name="hsb", bufs=2) as hsbpool,
            tc.tile_pool(name="e8", bufs=2) as epool,
            tc.tile_pool(name="oh", bufs=16) as ohpool,
            tc.tile_pool(name="outsb", bufs=2) as outpool,
            tc.tile_pool(name="acc", bufs=1, space="PSUM") as accpool,
            tc.tile_pool(name="hps", bufs=1, space="PSUM") as hpspool,
            tc.tile_pool(name="xtps", bufs=2, space="PSUM") as xtpspool,
            tc.tile_pool(name="lg", bufs=1, space="PSUM") as lgpool,
        ):
            # ---- constants into SBUF
            w1a_sb = cpool.tile([P, H], BF16, tag="w1a")
            w1b_sb = cpool.tile([P, H], BF16, tag="w1b")
            b1_sb = cpool.tile([H, 1], F32, tag="b1")
            w2_sb = cpool.tile([H, 1], BF16, tag="w2")
            id_sb = cpool.tile([P, P], BF16, tag="ident")
            io_sb = cpool.tile([P, P], BF16, tag="iota")
            ones_sb = cpool.tile([P, 1], BF16, tag="ones")
            blr_sb = cpool.tile([P, ch], F32, tag="blr")
            nc.sync.dma_start(w1a_sb[:], w1[0:P, :])
            nc.sync.dma_start(w1b_sb[:], w1[P : 2 * P, :])
            nc.sync.dma_start(b1_sb[:], b1[:])
            nc.sync.dma_start(w2_sb[:], w2[:])
            nc.sync.dma_start(id_sb[:], ident[:])
            nc.sync.dma_start(io_sb[:], iota[:])
            nc.sync.dma_start(blr_sb[:], blr[:])
            nc.vector.memset(ones_sb[:], 1.0)

            # ---- persistent accumulators (PSUM)
            pooled01 = accpool.tile([P, 2 * D], F32, tag="p01")  # blocks 0,1
            pooled23 = accpool.tile([P, 2 * D], F32, tag="p23")  # blocks 2,3
            den = accpool.tile([P, NBLK], F32, tag="den")

            xs3 = xs.rearrange("(g j p) d -> g j p d", j=8, p=P)  # [n_g8,8,128,256]

            def flush_pooled(items):
                # pooled[g] += oh.T @ x ; denom[g] += oh.T @ 1 (one group late
                # so PE has transpose/W1 work between logits and pooled mms)
                for oh, xbt, j8, c in items:
                    b = c // cpb
                    first = c == b * cpb
                    last = c == (b + 1) * cpb - 1
                    ptile = pooled01 if b < 2 else pooled23
                    col = (b % 2) * D
                    nc.tensor.matmul(
                        ptile[:, col : col + D],
                        oh[:],
                        xbt[:, j8 * D : (j8 + 1) * D],
                        start=first,
                        stop=last,
                    )
                    nc.tensor.matmul(
                        den[:, b : b + 1],
                        oh[:],
                        ones_sb[:],
                        start=first,
                        stop=last,
                    )

            pending = []
            for g8 in range(n_g8):
                xb = xbpool.tile([P, 8 * D], BF16)
                nc.sync.dma_start(
                    xb[:].rearrange("p (j d) -> p j d", j=8),
                    xs3[g8].rearrange("j p d -> p j d"),
                )
                lg = lgpool.tile([P, 8], F32)
                for half in range(2):
                    # two hps banks so chunk pairs can interleave their
                    # accumulation groups -> W1a/W1b each load once per pair
                    hpsA = hpspool.tile([P, 2 * H], F32, tag="hpsA")
                    hpsB = hpspool.tile([P, 2 * H], F32, tag="hpsB")
                    xt_ps = xtpspool.tile([P, 4 * D], BF16)
                    for k in range(4):
                        j8 = half * 4 + k
                        nc.tensor.transpose(
                            xt_ps[:, k * D : k * D + P],
                            xb[:, j8 * D : j8 * D + P],
                            id_sb[:],
                        )
                        nc.tensor.transpose(
                            xt_ps[:, k * D + P : (k + 1) * D],
                            xb[:, j8 * D + P : (j8 + 1) * D],
                            id_sb[:],
                        )
                    xt_sb = xtsbpool.tile([P, 4 * D], BF16)
                    nc.vector.tensor_copy(xt_sb[:], xt_ps[:])
                    # hT[j, i] += W1[d, j]^T · xT[d, i]; chunks k0/k1 go to
                    # different banks (groups in one bank must not interleave)
                    for pair in range(2):
                        k0, k1 = 2 * pair, 2 * pair + 1
                        col = pair * H
                        nc.tensor.matmul(
                            hpsA[:, col : col + H], w1a_sb[:],
                            xt_sb[:, k0 * D : k0 * D + P],
                            start=True, stop=False,
                        )
                        nc.tensor.matmul(
                            hpsB[:, col : col + H], w1a_sb[:],
                            xt_sb[:, k1 * D : k1 * D + P],
                            start=True, stop=False,
                        )
                        nc.tensor.matmul(
                            hpsA[:, col : col + H], w1b_sb[:],
                            xt_sb[:, k0 * D + P : (k0 + 1) * D],
                            start=False, stop=True,
                        )
                        nc.tensor.matmul(
                            hpsB[:, col : col + H], w1b_sb[:],
                            xt_sb[:, k1 * D + P : (k1 + 1) * D],
                            start=False, stop=True,
                        )
                    hsbA = hsbpool.tile([P, 2 * H], BF16, tag="hsbA")
                    hsbB = hsbpool.tile([P, 2 * H], BF16, tag="hsbB")
                    nc.scalar.activation(
                        hsbA[:], hpsA[:],
                        mybir.ActivationFunctionType.Tanh, bias=b1_sb[:],
                    )
                    nc.scalar.activation(
                        hsbB[:], hpsB[:],
                        mybir.ActivationFunctionType.Tanh, bias=b1_sb[:],
                    )
                    if DEBUG_TAPS and g8 == 0 and half == 0:
                        nc.sync.dma_start(dbg["xb"], xb[:])
                        nc.sync.dma_start(dbg["xt"], xt_sb[:])
                    for k in range(4):
                        j8 = half * 4 + k
                        hsb = hsbA if k % 2 == 0 else hsbB
                        col = (k // 2) * H
                        # logits[i] = hT[:, i]^T · W2  -> [128, 1]
                        nc.tensor.matmul(
                            lg[:, j8 : j8 + 1],
                            hsb[:, col : col + H],
                            w2_sb[:],
                            start=True,
                            stop=True,
                        )
                    if half == 0 and pending:
                        flush_pooled(pending)
                        pending = []
                e8 = epool.tile([P, 8], F32)
                nc.scalar.activation(
                    e8[:], lg[:], mybir.ActivationFunctionType.Exp
                )
                if DEBUG_TAPS and g8 == 0:
                    nc.sync.dma_start(dbg["e"], e8[:])
                    lgc = epool.tile([P, 8], F32, tag="lgc")
                    nc.vector.tensor_copy(lgc[:], lg[:])
                    nc.sync.dma_start(dbg["lg"], lgc[:])
                for j8 in range(8):
                    c = g8 * 8 + j8
                    oh = ohpool.tile([P, P], BF16)
                    # oh[i, g] = (iota[g] == rel_gid[i]) * e[i]
                    nc.vector.tensor_scalar(
                        oh[:],
                        io_sb[:],
                        blr_sb[:, c : c + 1],
                        e8[:, j8 : j8 + 1],
                        mybir.AluOpType.is_equal,
                        mybir.AluOpType.mult,
                    )
                    if DEBUG_TAPS and c == 0:
                        nc.sync.dma_start(dbg["oh"], oh[:])
                    pending.append((oh, xb, j8, c))
            flush_pooled(pending)
            pending = []

            # ---- epilogue: out[g] = pooled[g] / max(denom[g], tiny)
            for b in range(NBLK):
                dmax = outpool.tile([P, 1], F32, tag="dmax")
                rec = outpool.tile([P, 1], F32, tag="rec")
                nc.vector.tensor_scalar_max(dmax[:], den[:, b : b + 1], 1e-30)
                nc.vector.reciprocal(rec[:], dmax[:])
                ptile = pooled01 if b < 2 else pooled23
                col = (b % 2) * D
                osb = outpool.tile([P, D], F32, tag="osb")
                nc.scalar.mul(osb[:], ptile[:, col : col + D], rec[:])
                nc.sync.dma_start(out[b * P : (b + 1) * P, :], osb[:])

    nc.compile()
    return nc


# ----------------------------------------------------------------- kernel()

def kernel(**inputs):
    global LAST_RESULT
    x = np.asarray(inputs["x"])
    batch = np.asarray(inputs["batch"])
    W1 = np.asarray(inputs["W1"])
    b1 = np.asarray(inputs["b1"])
    W2 = np.asarray(inputs["W2"])
    # b2 cancels in the softmax; unused.

    seg, counts, cpb, ch = make_plan(batch)
    in_maps = build_inputs(x, batch, W1, b1, W2, seg, cpb, ch)
    nc = build_bass(ch, cpb)
    res = bass_utils.run_bass_kernel_spmd(
        nc, in_maps, list(range(NCORES))
    )
    LAST_RESULT = res
    out = np.concatenate(
        [np.asarray(res.results[c]["out"]) for c in range(NCORES)], axis=0
    )
    return out.astype(np.float32)

